# revision 3
# baseline (speedup 1.0000x reference)
# MemN2N forward kernel for Trainium2 (8 NeuronCores, Bass/Tile).
#
# Problem: B=256, V=50000, E=512, S=3 sentence slots, M=200 memories,
# HOPS=3, C=7 classes, D=S*E=1536.
#
# Sharding: data-parallel over batch, 32 batches per core. The embedding
# table is replicated; per core it is compacted to the tokens that core
# actually uses (so gather indices fit in int16 for dma_gather), pre-scaled
# by the (deterministic) position encoding and quantized to fp8e4 (x64),
# one table per sentence slot.
#
# Algorithm (per batch b):
#   m  = emb[stories_b] * enc          (200, 1536)  -- the expensive gather
#   u0 = emb[queries_b] * enc          (1536,)
#   mt = [m; u0]                       (201, 1536)  fp8, scaled by 64
#   Gram matrix G = mt @ mt.T (201x201, in 4096*units) contains every
#   attention inner product the 3 hops need:
#     dotted_0   = G[200, :200]                 (= m @ u0)
#     dotted_h+1 = dotted_h + G[:200,:200] @ p_h
#   The logits path stays accurate via F = [m;u0] @ fc_w.T computed from a
#   host-precomputed per-token table (f_s = emb*enc_s @ fc_w_s.T, exact
#   f32->bf16), DMA'd as 8 extra bf16 columns of the same hop operand:
#     y = F[200,:] + (p0+p1+p2) @ F[:200,:] + fc_b
#   so fp8 quantization only perturbs softmax scores (negligible), never
#   the logits directly.
#
# On device, a PSUM scores tile S[32, 208] accumulates, per batch row b,
#   (e_200 + p0 + p1 + p2) @ [G | F]_b
# via matmuls whose stationary operand is a [K, 32] matrix with only
# column b nonzero (diagonal-embedded p vectors), which lets all 32
# batches share one PSUM tile, keeps softmax batched, and makes the
# final logits fall out of PSUM columns 200..206.
#
# The Gram matmuls run in fp8 DoubleRow perf mode (2 fp8 MACs per PE
# cell): the transposed dma_gather writes 16-bit units u=(2d,2d+1) of
# each row to partition u%128, chunk u//128, so the gathered tile viewed
# as bytes is mt[p, cu, 2*i+k] = row_i[2*(cu*128+p)+k]. Pairing the
# contraction over cu (AP step 2*NIDX, %16==0 per the ISA restriction)
# gives two DoubleRow matmuls per slot (k=0,1) that together cover all
# 512 dims.

import numpy as np
import ml_dtypes

# ---- problem constants (hardcoded; kernel.py must be self-contained) ----
B, V, E, S, M, HOPS, C = 256, 50000, 512, 3, 200, 3, 7
D = S * E                   # 1536
NCORES = 8
BL = B // NCORES            # 32 batches per core
GB = 4                      # batches per gather group
NG = BL // GB               # 8 groups
NR = M + 1                  # 201 rows of the extended system [m; u0]
# gather indices per (group, slot): GB*NR rounded up to a multiple of 128.
# (transposed dma_gather is limited to ~1024 indices regardless of dtype;
# 1664 hangs the device even in fp8.)
NIDX = (GB * NR + 127) // 128 * 128     # 896
NLO = NR - 128              # 73 rows in the low Gram block
NCOL = M + 8                # 208 cols: 200 attention scores + 8 F columns
NQUEUES = 4                 # SWDGE queues for gather descriptor generation
SCALE = 64.0                # fp8 table scale; Gram lands in SCALE^2 units
SC2INV = float(2.0 ** -12)  # 1/SCALE^2, folded into the softmax exp

BF16 = ml_dtypes.bfloat16
FP8 = ml_dtypes.float8_e4m3

_CACHE = {}


def _position_encoding(sentence_size, embedding_size):
    i = np.arange(1, embedding_size + 1, dtype=np.float32)[:, None]
    j = np.arange(1, sentence_size + 1, dtype=np.float32)[None, :]
    le, ls = embedding_size + 1, sentence_size + 1
    enc = (i - (le - 1) / 2.0) * (j - (ls - 1) / 2.0)
    enc = 1.0 + 4.0 * enc / embedding_size / sentence_size
    return np.transpose(enc).astype(np.float32)


def _build_program(dpad, stage="full"):
    import concourse.bacc as bacc
    import concourse.bass as bass
    import concourse.mybir as mybir
    import concourse.tile as tile
    from concourse.masks import make_identity

    dt = mybir.dt
    nc = bacc.Bacc("TRN2", target_bir_lowering=False, debug=False,
                   num_swdge_queues=NQUEUES)
    dbg_t = None
    if stage != "full":
        dbg_t = nc.dram_tensor("dbg", [128, BL, NCOL], dt.float32,
                               kind="ExternalOutput")

    emb_t = [
        nc.dram_tensor(f"emb{s}", [dpad, E], dt.float8e4, kind="ExternalInput")
        for s in range(S)
    ]
    idxm_t = nc.dram_tensor("idxm", [128, NG * S, NIDX // 16], dt.int16,
                            kind="ExternalInput")
    fcb_t = nc.dram_tensor("fcb", [BL, C], dt.float32, kind="ExternalInput")
    e1m_t = nc.dram_tensor("e1m", [NLO, 32 * 32], dt.bfloat16,
                           kind="ExternalInput")
    fh_t = nc.dram_tensor("fh", [128, BL, 8], dt.bfloat16,
                          kind="ExternalInput")
    fl_t = nc.dram_tensor("fl", [NLO, BL, 8], dt.bfloat16,
                          kind="ExternalInput")
    y_t = nc.dram_tensor("y", [BL, C], dt.float32, kind="ExternalOutput")

    with tile.TileContext(nc) as tc:
        with (
            tc.tile_pool(name="const", bufs=1) as cpool,
            tc.tile_pool(name="gath", bufs=2) as gpool,
            tc.tile_pool(name="gram", bufs=1) as grpool,
            tc.tile_pool(name="work", bufs=2) as wpool,
            tc.tile_pool(name="psum", bufs=2, space="PSUM") as ppool,
            tc.tile_pool(name="psT", bufs=1, space="PSUM") as tpool,
            tc.tile_pool(name="psS", bufs=1, space="PSUM") as spool,
        ):
            # ---- constants / small inputs ----
            idm = cpool.tile([128, NG * S, NIDX // 16], dt.int16)
            nc.sync.dma_start(idm[:], idxm_t[:])
            fcb = cpool.tile([BL, C], dt.float32)
            nc.sync.dma_start(fcb[:], fcb_t[:])

            ident = cpool.tile([32, 32], dt.bfloat16)
            make_identity(nc, ident[:])

            # e200 selector: [NLO, 32*32] with [72, b*33] = 1 -> stationary
            # operand that routes [G|F]_b[200, :] into scores row b.
            e1m = cpool.tile([NLO, 32 * 32], dt.bfloat16)
            nc.sync.dma_start(e1m[:], e1m_t[:])

            # hop operand [G | F]: cols 0:200 Gram (written from PSUM per
            # batch), cols 200:208 host-exact F values (one DMA).
            grh = grpool.tile([128, BL, NCOL], dt.bfloat16)
            grl = grpool.tile([NLO, BL, NCOL], dt.bfloat16)
            nc.sync.dma_start(grh[:, :, M:NCOL], fh_t[:])
            nc.sync.dma_start(grl[:, :, M:NCOL], fl_t[:])

            # ---- main pipeline: gather group -> Gram ----
            for g in range(NG):
                mts = []
                for s in range(S):
                    mt = gpool.tile([128, 4, NIDX], dt.float8e4, tag=f"mt{s}")
                    nc.gpsimd.dma_gather(
                        mt[:],
                        emb_t[s][:, :],
                        idm[:, g * S + s, :],
                        NIDX, NIDX, E,
                        transpose=True,
                        queue_num=(g * S + s) % NQUEUES,
                    )
                    mts.append(mt)

                if stage == "gather":
                    if g == 0:
                        t = mts[0][:]
                        view = bass.AP(
                            t.tensor, t.offset,
                            [t.ap[0], t.ap[1], [NR, GB], [1, NR]],
                        )
                        dbgs = wpool.tile([128, 4, GB, NR], dt.float32,
                                          tag="dbgs")
                        nc.vector.tensor_copy(dbgs[:], view)
                        dv = dbgs[:].rearrange("p c b r -> p (c b r)")
                        nc.sync.dma_start(
                            dbg_t[:].rearrange("p a b -> p (a b)")[
                                :, 0:4 * GB * NR], dv)
                    continue

                for b8 in range(GB):
                    bg = g * GB + b8
                    ph = ppool.tile([128, M], dt.float32, tag="ph")
                    pl = ppool.tile([NLO, M], dt.float32, tag="pl")
                    for s in range(S):
                        t = mts[s][:]
                        for k in range(2):
                            ki = 2 * s + k
                            off = t.offset + (b8 * NR) * 2 + k
                            lhsT_h = bass.AP(
                                t.tensor, off,
                                [t.ap[0], [2 * NIDX, 2], [2, 128]])
                            lhsT_l = bass.AP(
                                t.tensor, off + 256,
                                [t.ap[0], [2 * NIDX, 2], [2, NLO]])
                            rhs = bass.AP(
                                t.tensor, off,
                                [t.ap[0], [2 * NIDX, 2], [2, M]])
                            nc.tensor.matmul(
                                ph[:], lhsT=lhsT_h, rhs=rhs,
                                start=(ki == 0), stop=(ki == 5),
                                perf_mode=mybir.MatmulPerfMode.DoubleRow,
                            )
                            nc.tensor.matmul(
                                pl[:], lhsT=lhsT_l, rhs=rhs,
                                start=(ki == 0), stop=(ki == 5),
                                perf_mode=mybir.MatmulPerfMode.DoubleRow,
                            )
                    nc.scalar.copy(grh[:, bg, 0:M], ph[:])
                    nc.scalar.copy(grl[:, bg, 0:M], pl[:])

            if stage == "gram":
                dbgs = wpool.tile([128, BL, NCOL], dt.float32, tag="dbgs")
                nc.vector.tensor_copy(dbgs[:], grh[:])
                nc.sync.dma_start(dbg_t[:], dbgs[:])

            # ---- hops ----
            do_hops = stage in ("full", "hops1")
            nhops = HOPS if stage == "full" else 1
            if do_hops:
                Sc = spool.tile([BL, NCOL], dt.float32)
                for b in range(BL):
                    nc.tensor.matmul(
                        Sc[:], lhsT=e1m[:, b * 32:(b + 1) * 32],
                        rhs=grl[:, b, :],
                        start=(b == 0), stop=False, skip_group_check=True,
                    )
            for h in range(nhops if do_hops else 0):
                eexp = wpool.tile([BL, M], dt.float32, tag="eexp")
                sume = wpool.tile([BL, 1], dt.float32, tag="sume")
                nc.scalar.activation(
                    eexp[:], Sc[:, 0:M],
                    mybir.ActivationFunctionType.Exp,
                    scale=SC2INV,
                    accum_out=sume[:],
                )
                rs = wpool.tile([BL, 1], dt.float32, tag="rs")
                nc.vector.reciprocal(rs[:], sume[:])
                pbf = wpool.tile([BL, M], dt.bfloat16, tag="pbf")
                nc.vector.tensor_scalar_mul(pbf[:], eexp[:], rs[:])

                pth = tpool.tile([128, 32], dt.bfloat16, tag="pth")
                ptl = tpool.tile([M - 128, 32], dt.bfloat16, tag="ptl")
                nc.tensor.transpose(pth[:], pbf[:, 0:128], ident[:])
                nc.tensor.transpose(ptl[:], pbf[:, 128:M], ident[:])

                pm0 = wpool.tile([128, 32 * 32], dt.bfloat16, tag="pm0")
                pm1 = wpool.tile([NLO, 32 * 32], dt.bfloat16, tag="pm1")
                nc.vector.memset(pm0[:], 0.0)
                nc.vector.memset(pm1[:], 0.0)
                nc.vector.tensor_copy(pm0[:, ::33], pth[:])
                nc.vector.tensor_copy(pm1[0:M - 128, ::33], ptl[:])

                last = h == nhops - 1
                for b in range(BL):
                    nc.tensor.matmul(
                        Sc[:], lhsT=pm0[:, b * 32:(b + 1) * 32],
                        rhs=grh[:, b, :],
                        start=False, stop=False, skip_group_check=True,
                    )
                    nc.tensor.matmul(
                        Sc[:], lhsT=pm1[:, b * 32:(b + 1) * 32],
                        rhs=grl[:, b, :],
                        start=False, stop=(last and b == BL - 1),
                        skip_group_check=True,
                    )

            yt = wpool.tile([BL, C], dt.float32, tag="yt")
            if do_hops:
                nc.vector.tensor_add(yt[:], Sc[:, M:M + C], fcb[:])
                if stage == "hops1":
                    dbgs = wpool.tile([128, BL, NCOL], dt.float32, tag="dbgs")
                    nc.vector.memset(dbgs[:], 0.0)
                    nc.vector.tensor_copy(dbgs[0:BL, 0, :], Sc[:])
                    nc.sync.dma_start(dbg_t[:], dbgs[:])
            else:
                nc.vector.memset(yt[:], 0.0)
            nc.sync.dma_start(y_t[:], yt[:])

    nc.compile()
    return nc


def _wrap16(lst):
    """Index list -> dma_gather layout: [16, n/16] with logical i at
    [i % 16, i // 16], replicated to 128 partitions."""
    a = np.asarray(lst, dtype=np.int16)
    assert a.size % 16 == 0
    a2 = a.reshape(-1, 16).T.copy()
    return np.tile(a2, (8, 1))


def _prepare_core_inputs(stories, queries, emb, fc_w, fc_b, enc):
    """Host-side shard prep: per-core token compaction + index layouts.

    Each per-slot table holds the enc-scaled, x64 fp8-quantized embedding
    rows for this core's tokens. The logits-path values F (= row @ fc_w.T)
    are precomputed per token in f32 (exact) and gathered on the host into
    small bf16 arrays loaded with a plain DMA."""
    per_core = []
    toks_list = []
    for cid in range(NCORES):
        st = stories[cid * BL:(cid + 1) * BL]
        qu = queries[cid * BL:(cid + 1) * BL]
        toks = np.unique(np.concatenate([st.ravel(), qu.ravel()]))
        toks_list.append(toks)
    dpad = max(len(t) for t in toks_list)
    dpad = (dpad + 127) // 128 * 128

    # full-vocab per-slot fp8 tables and exact F tables (vectorized)
    emb8 = []
    fs = []
    for s in range(S):
        sc = emb * enc[s * E:(s + 1) * E][None, :]
        emb8.append((sc * SCALE).astype(FP8))
        fs.append((sc @ fc_w[:, s * E:(s + 1) * E].T).astype(np.float32))

    fcb_rep = np.tile(fc_b[None, :], (BL, 1)).astype(np.float32)
    e1m = np.zeros((NLO, 32 * 32), dtype=BF16)
    e1m[NR - 1 - 128, ::33] = 1.0

    for cid in range(NCORES):
        st = stories[cid * BL:(cid + 1) * BL]     # (BL, M, S)
        qu = queries[cid * BL:(cid + 1) * BL]     # (BL, S)
        toks = toks_list[cid]
        ntok = len(toks)
        inv = np.zeros(V, dtype=np.int64)
        inv[toks] = np.arange(ntok)

        embs = []
        for s in range(S):
            tbl = np.zeros((dpad, E), dtype=FP8)
            tbl[:ntok] = emb8[s][toks]
            embs.append(tbl)

        sidx = inv[st]          # (BL, M, S)
        qidx = inv[qu]          # (BL, S)

        idxm = np.zeros((128, NG * S, NIDX // 16), dtype=np.int16)
        for g in range(NG):
            for s in range(S):
                lst = np.zeros(NIDX, dtype=np.int64)
                blk = lst[:GB * NR].reshape(GB, NR)
                blk[:, :M] = sidx[g * GB:(g + 1) * GB, :, s]
                blk[:, M] = qidx[g * GB:(g + 1) * GB, s]
                idxm[:, g * S + s, :] = _wrap16(lst)

        # F = [m; u0] @ fc_w.T per batch, exact f32 -> bf16, [row, BL, 8]
        fstory = sum(fs[s][st[:, :, s]] for s in range(S))   # (BL, M, C)
        fquery = sum(fs[s][qu[:, s]] for s in range(S))      # (BL, C)
        fh = np.zeros((128, BL, 8), dtype=BF16)
        fl = np.zeros((NLO, BL, 8), dtype=BF16)
        fh[:, :, :C] = fstory[:, 0:128, :].transpose(1, 0, 2)
        fl[0:M - 128, :, :C] = fstory[:, 128:M, :].transpose(1, 0, 2)
        fl[M - 128, :, :C] = fquery

        in_map = {
            "emb0": embs[0], "emb1": embs[1], "emb2": embs[2],
            "idxm": idxm, "fcb": fcb_rep, "e1m": e1m,
            "fh": fh, "fl": fl,
        }
        per_core.append(in_map)
    return dpad, per_core


def kernel(stories, queries, emb, fc_w, fc_b, _trace=False):
    from concourse import bass_utils

    stories = np.asarray(stories)
    queries = np.asarray(queries)
    emb = np.asarray(emb, dtype=np.float32)
    fc_w = np.asarray(fc_w, dtype=np.float32)
    fc_b = np.asarray(fc_b, dtype=np.float32)

    enc = _position_encoding(1, D).reshape(D)
    dpad, in_maps = _prepare_core_inputs(stories, queries, emb, fc_w, fc_b, enc)

    if _CACHE.get("dpad") != dpad:
        _CACHE["nc"] = _build_program(dpad)
        _CACHE["dpad"] = dpad
    nc = _CACHE["nc"]

    res = bass_utils.run_bass_kernel_spmd(
        nc, in_maps, core_ids=list(range(NCORES)), trace=_trace,
    )
    out = np.concatenate([r["y"] for r in res.results], axis=0)
    if _trace:
        _CACHE["last_exec_time_ns"] = res.exec_time_ns
        _CACHE["last_mean_exec_time_ns"] = res.mean_exec_time_ns
    return out.astype(np.float32)


# revision 11
# speedup vs baseline: 1.0819x; 1.0819x over previous
# MemN2N forward kernel for Trainium2 (8 NeuronCores, Bass/Tile).
#
# Problem: B=256, V=50000, E=512, S=3 sentence slots, M=200 memories,
# HOPS=3, C=7 classes, D=S*E=1536.
#
# Sharding: data-parallel over batch, 32 batches per core. The embedding
# table is replicated; per core it is compacted to the tokens that core
# actually uses (so gather indices fit in int16 for dma_gather), pre-scaled
# by the (deterministic) position encoding and quantized to fp8e4 (x64),
# one table per sentence slot.
#
# Algorithm (per batch b):
#   m  = emb[stories_b] * enc          (200, 1536)  -- the expensive gather
#   u0 = emb[queries_b] * enc          (1536,)
#   mt = [m; u0]                       (201, 1536)  fp8, scaled by 64
#   Gram matrix G = mt @ mt.T (201x201, in 4096*units) contains every
#   attention inner product the 3 hops need:
#     dotted_0   = G[200, :200]                 (= m @ u0)
#     dotted_h+1 = dotted_h + G[:200,:200] @ p_h
#   The logits path stays accurate via F = [m;u0] @ fc_w.T computed from a
#   host-precomputed per-token table (f_s = emb*enc_s @ fc_w_s.T, exact
#   f32->bf16), DMA'd as 8 extra bf16 columns of the same hop operand:
#     y = F[200,:] + (p0+p1+p2) @ F[:200,:] + fc_b
#   so fp8 quantization only perturbs softmax scores (negligible), never
#   the logits directly.
#
# On device, a PSUM scores tile S[32, 208] accumulates, per batch row b,
#   (e_200 + p0 + p1 + p2) @ [G | F]_b
# via matmuls whose stationary operand is a [K, 32] matrix with only
# column b nonzero (diagonal-embedded p vectors), which lets all 32
# batches share one PSUM tile, keeps softmax batched, and makes the
# final logits fall out of PSUM columns 200..206.
#
# The Gram matmuls run in fp8 DoubleRow perf mode (2 fp8 MACs per PE
# cell): the transposed dma_gather writes 16-bit units u=(2d,2d+1) of
# each row to partition u%128, chunk u//128, so the gathered tile viewed
# as bytes is mt[p, cu, 2*i+k] = row_i[2*(cu*128+p)+k]. Pairing the
# contraction over cu (AP step 2*NIDX, %16==0 per the ISA restriction)
# gives two DoubleRow matmuls per slot (k=0,1) that together cover all
# 512 dims.

import numpy as np
import ml_dtypes

# ---- problem constants (hardcoded; kernel.py must be self-contained) ----
B, V, E, S, M, HOPS, C = 256, 50000, 512, 3, 200, 3, 7
D = S * E                   # 1536
NCORES = 8
BL = B // NCORES            # 32 batches per core
GB = 4                      # batches per gather group
NG = BL // GB               # 8 groups
NR = M + 1                  # 201 rows of the extended system [m; u0]
# gather indices per (group, slot): GB*NR rounded up to a multiple of 128.
# (transposed dma_gather is limited to ~1024 indices regardless of dtype;
# 1664 hangs the device even in fp8.)
NIDX = (GB * NR + 127) // 128 * 128     # 896
NLO = NR - 128              # 73 rows in the low Gram block
NCOL = M + 8                # 208 cols: 200 attention scores + 8 F columns
NQUEUES = 4                 # SWDGE queues for gather descriptor generation
SCALE = 64.0                # fp8 table scale; Gram lands in SCALE^2 units
SC2INV = float(2.0 ** -12)  # 1/SCALE^2, folded into the softmax exp

BF16 = ml_dtypes.bfloat16
FP8 = ml_dtypes.float8_e4m3

_CACHE = {}


def _position_encoding(sentence_size, embedding_size):
    i = np.arange(1, embedding_size + 1, dtype=np.float32)[:, None]
    j = np.arange(1, sentence_size + 1, dtype=np.float32)[None, :]
    le, ls = embedding_size + 1, sentence_size + 1
    enc = (i - (le - 1) / 2.0) * (j - (ls - 1) / 2.0)
    enc = 1.0 + 4.0 * enc / embedding_size / sentence_size
    return np.transpose(enc).astype(np.float32)


def _build_program(dpad, stage="full"):
    import concourse.bacc as bacc
    import concourse.bass as bass
    import concourse.mybir as mybir
    import concourse.tile as tile
    from concourse.masks import make_identity

    dt = mybir.dt
    nc = bacc.Bacc("TRN2", target_bir_lowering=False, debug=False,
                   num_swdge_queues=NQUEUES)
    dbg_t = None
    if stage != "full":
        dbg_t = nc.dram_tensor("dbg", [128, BL, NCOL], dt.float32,
                               kind="ExternalOutput")

    emb_t = [
        nc.dram_tensor(f"emb{s}", [dpad, E], dt.float8e4, kind="ExternalInput")
        for s in range(S)
    ]
    idxm_t = nc.dram_tensor("idxm", [128, NG * S, NIDX // 16], dt.int16,
                            kind="ExternalInput")
    fcb_t = nc.dram_tensor("fcb", [BL, C], dt.float32, kind="ExternalInput")
    e1m_t = nc.dram_tensor("e1m", [NLO, 32 * 32], dt.bfloat16,
                           kind="ExternalInput")
    fh_t = nc.dram_tensor("fh", [128, BL * 8], dt.bfloat16,
                          kind="ExternalInput")
    fl_t = nc.dram_tensor("fl", [NLO, BL * 8], dt.bfloat16,
                          kind="ExternalInput")
    y_t = nc.dram_tensor("y", [BL, C], dt.float32, kind="ExternalOutput")

    with tile.TileContext(nc) as tc:
        with (
            tc.tile_pool(name="const", bufs=1) as cpool,
            tc.tile_pool(name="gath", bufs=2) as gpool,
            tc.tile_pool(name="gram", bufs=1) as grpool,
            tc.tile_pool(name="work", bufs=2) as wpool,
            tc.tile_pool(name="psum", bufs=2, space="PSUM") as ppool,
            tc.tile_pool(name="psT", bufs=1, space="PSUM") as tpool,
            tc.tile_pool(name="psS", bufs=1, space="PSUM") as spool,
        ):
            # ---- constants / small inputs ----
            idm = cpool.tile([128, NG * S, NIDX // 16], dt.int16)
            nc.sync.dma_start(idm[:], idxm_t[:])
            fcb = cpool.tile([BL, C], dt.float32)
            nc.sync.dma_start(fcb[:], fcb_t[:])

            ident = cpool.tile([32, 32], dt.bfloat16)
            make_identity(nc, ident[:])

            # e200 selector: [NLO, 32*32] with [72, b*33] = 1 -> stationary
            # operand that routes [G|F]_b[200, :] into scores row b.
            e1m = cpool.tile([NLO, 32 * 32], dt.bfloat16)
            nc.sync.dma_start(e1m[:], e1m_t[:])

            # hop operand [G | F]: cols 0:200 Gram (written from PSUM per
            # batch), cols 200:208 host-exact F values. The F values arrive
            # via a contiguous DMA into a staging tile (a strided dram->sbuf
            # DMA decomposes into thousands of 16B descriptors and poisons
            # the rings) and a single strided DVE copy.
            grh = grpool.tile([128, BL, NCOL], dt.bfloat16)
            grl = grpool.tile([NLO, BL, NCOL], dt.bfloat16)
            fhs = cpool.tile([128, BL * 8], dt.bfloat16)
            fls = cpool.tile([NLO, BL * 8], dt.bfloat16)
            nc.sync.dma_start(fhs[:], fh_t[:])
            nc.sync.dma_start(fls[:], fl_t[:])
            nc.vector.tensor_copy(
                grh[:, :, M:NCOL], fhs[:].rearrange("p (b f) -> p b f", f=8))
            nc.vector.tensor_copy(
                grl[:, :, M:NCOL], fls[:].rearrange("p (b f) -> p b f", f=8))

            Sc = spool.tile([BL, NCOL], dt.float32)

            # ---- main pipeline: gather group -> Gram ----
            for g in range(NG):
                mts = []
                for s in range(S):
                    mt = gpool.tile([128, 4, NIDX], dt.float8e4, tag=f"mt{s}")
                    nc.gpsimd.dma_gather(
                        mt[:],
                        emb_t[s][:, :],
                        idm[:, g * S + s, :],
                        NIDX, GB * NR, E,
                        transpose=True,
                        queue_num=(g * S + s) % NQUEUES,
                    )
                    mts.append(mt)

                if stage == "gather":
                    if g == 0:
                        t = mts[0][:]
                        view = bass.AP(
                            t.tensor, t.offset,
                            [t.ap[0], t.ap[1], [NR, GB], [1, NR]],
                        )
                        dbgs = wpool.tile([128, 4, GB, NR], dt.float32,
                                          tag="dbgs")
                        nc.vector.tensor_copy(dbgs[:], view)
                        dv = dbgs[:].rearrange("p c b r -> p (c b r)")
                        nc.sync.dma_start(
                            dbg_t[:].rearrange("p a b -> p (a b)")[
                                :, 0:4 * GB * NR], dv)
                    continue

                for b8 in range(GB):
                    bg = g * GB + b8
                    ph = ppool.tile([128, M], dt.float32, tag="ph")
                    pl = ppool.tile([NLO, M], dt.float32, tag="pl")
                    for s in range(S):
                        t = mts[s][:]
                        for k in range(2):
                            ki = 2 * s + k
                            off = t.offset + (b8 * NR) * 2 + k
                            lhsT_h = bass.AP(
                                t.tensor, off,
                                [t.ap[0], [2 * NIDX, 2], [2, 128]])
                            lhsT_l = bass.AP(
                                t.tensor, off + 256,
                                [t.ap[0], [2 * NIDX, 2], [2, NLO]])
                            rhs = bass.AP(
                                t.tensor, off,
                                [t.ap[0], [2 * NIDX, 2], [2, M]])
                            nc.tensor.matmul(
                                ph[:], lhsT=lhsT_h, rhs=rhs,
                                start=(ki == 0), stop=(ki == 5),
                                perf_mode=mybir.MatmulPerfMode.DoubleRow,
                            )
                            nc.tensor.matmul(
                                pl[:], lhsT=lhsT_l, rhs=rhs,
                                start=(ki == 0), stop=(ki == 5),
                                perf_mode=mybir.MatmulPerfMode.DoubleRow,
                            )
                    nc.scalar.copy(grh[:, bg, 0:M], ph[:])
                    nc.scalar.copy(grl[:, bg, 0:M], pl[:])
                    if stage in ("full", "hops1"):
                        # fold the e200 init matmul into the Gram pipeline:
                        # scores row bg = [G|F]_bg[200, :] (batch bg only).
                        nc.tensor.matmul(
                            Sc[:], lhsT=e1m[:, bg * 32:(bg + 1) * 32],
                            rhs=grl[:, bg, :],
                            start=(bg == 0), stop=False,
                            skip_group_check=True,
                        )

            if stage == "gram":
                dbgs = wpool.tile([128, BL, NCOL], dt.float32, tag="dbgs")
                nc.vector.tensor_copy(dbgs[:], grh[:])
                nc.sync.dma_start(dbg_t[:], dbgs[:])

            # ---- hops ----
            do_hops = stage in ("full", "hops1")
            nhops = HOPS if stage == "full" else 1
            for h in range(nhops if do_hops else 0):
                eexp = wpool.tile([BL, M], dt.float32, tag="eexp")
                sume = wpool.tile([BL, 1], dt.float32, tag="sume")
                nc.scalar.activation(
                    eexp[:], Sc[:, 0:M],
                    mybir.ActivationFunctionType.Exp,
                    scale=SC2INV,
                    accum_out=sume[:],
                )
                rs = wpool.tile([BL, 1], dt.float32, tag="rs")
                nc.vector.reciprocal(rs[:], sume[:])
                pbf = wpool.tile([BL, M], dt.bfloat16, tag="pbf")
                nc.vector.tensor_scalar_mul(pbf[:], eexp[:], rs[:])

                pth = tpool.tile([128, 32], dt.bfloat16, tag="pth")
                ptl = tpool.tile([M - 128, 32], dt.bfloat16, tag="ptl")
                nc.tensor.transpose(pth[:], pbf[:, 0:128], ident[:])
                nc.tensor.transpose(ptl[:], pbf[:, 128:M], ident[:])

                pm0 = wpool.tile([128, 32 * 32], dt.bfloat16, tag="pm0")
                pm1 = wpool.tile([NLO, 32 * 32], dt.bfloat16, tag="pm1")
                nc.vector.memset(pm0[:], 0.0)
                nc.vector.memset(pm1[:], 0.0)
                nc.vector.tensor_copy(pm0[:, ::33], pth[:])
                nc.vector.tensor_copy(pm1[0:M - 128, ::33], ptl[:])

                last = h == nhops - 1
                for b in range(BL):
                    nc.tensor.matmul(
                        Sc[:], lhsT=pm0[:, b * 32:(b + 1) * 32],
                        rhs=grh[:, b, :],
                        start=False, stop=False, skip_group_check=True,
                    )
                    nc.tensor.matmul(
                        Sc[:], lhsT=pm1[:, b * 32:(b + 1) * 32],
                        rhs=grl[:, b, :],
                        start=False, stop=(last and b == BL - 1),
                        skip_group_check=True,
                    )

            yt = wpool.tile([BL, C], dt.float32, tag="yt")
            if do_hops:
                nc.vector.tensor_add(yt[:], Sc[:, M:M + C], fcb[:])
                if stage == "hops1":
                    dbgs = wpool.tile([128, BL, NCOL], dt.float32, tag="dbgs")
                    nc.vector.memset(dbgs[:], 0.0)
                    nc.vector.tensor_copy(dbgs[0:BL, 0, :], Sc[:])
                    nc.sync.dma_start(dbg_t[:], dbgs[:])
            else:
                nc.vector.memset(yt[:], 0.0)
            nc.sync.dma_start(y_t[:], yt[:])

    nc.compile()
    return nc


def _wrap16(lst):
    """Index list -> dma_gather layout: [16, n/16] with logical i at
    [i % 16, i // 16], replicated to 128 partitions."""
    a = np.asarray(lst, dtype=np.int16)
    assert a.size % 16 == 0
    a2 = a.reshape(-1, 16).T.copy()
    return np.tile(a2, (8, 1))


def _prepare_core_inputs(stories, queries, emb, fc_w, fc_b, enc):
    """Host-side shard prep: per-core token compaction + index layouts.

    Each per-slot table holds the enc-scaled, x64 fp8-quantized embedding
    rows for this core's tokens. The logits-path values F (= row @ fc_w.T)
    are precomputed per token in f32 (exact) and gathered on the host into
    small bf16 arrays loaded with a plain DMA."""
    per_core = []
    toks_list = []
    for cid in range(NCORES):
        st = stories[cid * BL:(cid + 1) * BL]
        qu = queries[cid * BL:(cid + 1) * BL]
        toks = np.unique(np.concatenate([st.ravel(), qu.ravel()]))
        toks_list.append(toks)
    dpad = max(len(t) for t in toks_list)
    dpad = (dpad + 127) // 128 * 128

    # full-vocab per-slot fp8 tables and exact F tables (vectorized)
    emb8 = []
    fs = []
    for s in range(S):
        sc = emb * enc[s * E:(s + 1) * E][None, :]
        emb8.append((sc * SCALE).astype(FP8))
        fs.append((sc @ fc_w[:, s * E:(s + 1) * E].T).astype(np.float32))

    fcb_rep = np.tile(fc_b[None, :], (BL, 1)).astype(np.float32)
    e1m = np.zeros((NLO, 32 * 32), dtype=BF16)
    e1m[NR - 1 - 128, ::33] = 1.0

    for cid in range(NCORES):
        st = stories[cid * BL:(cid + 1) * BL]     # (BL, M, S)
        qu = queries[cid * BL:(cid + 1) * BL]     # (BL, S)
        toks = toks_list[cid]
        ntok = len(toks)
        inv = np.zeros(V, dtype=np.int64)
        inv[toks] = np.arange(ntok)

        embs = []
        for s in range(S):
            tbl = np.zeros((dpad, E), dtype=FP8)
            tbl[:ntok] = emb8[s][toks]
            embs.append(tbl)

        sidx = inv[st]          # (BL, M, S)
        qidx = inv[qu]          # (BL, S)

        idxm = np.zeros((128, NG * S, NIDX // 16), dtype=np.int16)
        for g in range(NG):
            for s in range(S):
                # pad with -1: the SWDGE stops after the last valid index
                # (num_idxs_reg = GB*NR), skipping the pad descriptors.
                lst = np.full(NIDX, -1, dtype=np.int64)
                blk = lst[:GB * NR].reshape(GB, NR)
                blk[:, :M] = sidx[g * GB:(g + 1) * GB, :, s]
                blk[:, M] = qidx[g * GB:(g + 1) * GB, s]
                idxm[:, g * S + s, :] = _wrap16(lst)

        # F = [m; u0] @ fc_w.T per batch, exact f32 -> bf16, [row, BL, 8]
        fstory = sum(fs[s][st[:, :, s]] for s in range(S))   # (BL, M, C)
        fquery = sum(fs[s][qu[:, s]] for s in range(S))      # (BL, C)
        fh = np.zeros((128, BL, 8), dtype=BF16)
        fl = np.zeros((NLO, BL, 8), dtype=BF16)
        fh[:, :, :C] = fstory[:, 0:128, :].transpose(1, 0, 2)
        fl[0:M - 128, :, :C] = fstory[:, 128:M, :].transpose(1, 0, 2)
        fl[M - 128, :, :C] = fquery
        fh = fh.reshape(128, BL * 8)
        fl = fl.reshape(NLO, BL * 8)

        in_map = {
            "emb0": embs[0], "emb1": embs[1], "emb2": embs[2],
            "idxm": idxm, "fcb": fcb_rep, "e1m": e1m,
            "fh": fh, "fl": fl,
        }
        per_core.append(in_map)
    return dpad, per_core


def kernel(stories, queries, emb, fc_w, fc_b, _trace=False):
    from concourse import bass_utils

    stories = np.asarray(stories)
    queries = np.asarray(queries)
    emb = np.asarray(emb, dtype=np.float32)
    fc_w = np.asarray(fc_w, dtype=np.float32)
    fc_b = np.asarray(fc_b, dtype=np.float32)

    enc = _position_encoding(1, D).reshape(D)
    dpad, in_maps = _prepare_core_inputs(stories, queries, emb, fc_w, fc_b, enc)

    if _CACHE.get("dpad") != dpad:
        _CACHE["nc"] = _build_program(dpad)
        _CACHE["dpad"] = dpad
    nc = _CACHE["nc"]

    res = bass_utils.run_bass_kernel_spmd(
        nc, in_maps, core_ids=list(range(NCORES)), trace=_trace,
    )
    out = np.concatenate([r["y"] for r in res.results], axis=0)
    if _trace:
        _CACHE["last_exec_time_ns"] = res.exec_time_ns
        _CACHE["last_mean_exec_time_ns"] = res.mean_exec_time_ns
    return out.astype(np.float32)


# revision 12
# speedup vs baseline: 1.1911x; 1.1009x over previous
# MemN2N forward kernel for Trainium2 (8 NeuronCores, Bass/Tile).
#
# Problem: B=256, V=50000, E=512, S=3 sentence slots, M=200 memories,
# HOPS=3, C=7 classes, D=S*E=1536.
#
# Sharding: data-parallel over batch, 32 batches per core. The embedding
# table is replicated; per core it is compacted to the tokens that core
# actually uses (so gather indices fit in int16 for dma_gather), pre-scaled
# by the (deterministic) position encoding and quantized to fp8e4 (x64),
# one table per sentence slot.
#
# Algorithm (per batch b):
#   m  = emb[stories_b] * enc          (200, 1536)  -- the expensive gather
#   u0 = emb[queries_b] * enc          (1536,)
#   mt = [m; u0]                       (201, 1536)  fp8, scaled by 64
#   Gram matrix G = mt @ mt.T (201x201, in 4096*units) contains every
#   attention inner product the 3 hops need:
#     dotted_0   = G[200, :200]                 (= m @ u0)
#     dotted_h+1 = dotted_h + G[:200,:200] @ p_h
#   The logits path stays accurate via F = [m;u0] @ fc_w.T computed from a
#   host-precomputed per-token table (f_s = emb*enc_s @ fc_w_s.T, exact
#   f32->bf16), loaded as 8 extra bf16 columns of the same hop operand:
#     y = F[200,:] + (p0+p1+p2) @ F[:200,:] + fc_b
#   so fp8 quantization only perturbs softmax scores (negligible), never
#   the logits directly.
#
# On device, per 16-batch cohort, a PSUM scores tile S[16, 208] accumulates
#   (e_200 + p0 + p1 + p2) @ [G | F]_b
# per batch row b, via matmuls whose stationary operand is a [K, 16] matrix
# with only column b nonzero (diagonal-embedded p vectors). Splitting the
# 32 batches into two cohorts lets cohort A's three (serial) hops run on
# the PE while cohort B's gathers/Gram are still streaming in.
#
# The Gram matmuls run in fp8 DoubleRow perf mode (2 fp8 MACs per PE
# cell): the transposed dma_gather writes 16-bit units u=(2d,2d+1) of
# each row to partition u%128, chunk u//128, so the gathered tile viewed
# as bytes is mt[p, cu, 2*i+k] = row_i[2*(cu*128+p)+k]. Pairing the
# contraction over cu (AP step 2*NIDX, %16==0 per the ISA restriction)
# gives two DoubleRow matmuls per slot (k=0,1) that together cover all
# 512 dims.
#
# Gathers are kept small (2 batches per gather, 512 indices) so each SWDGE
# queue ring holds several gathers' descriptors and the DMA rings stay
# continuously fed (one big gather per queue causes drain/refill bubbles).

import numpy as np
import ml_dtypes

# ---- problem constants (hardcoded; kernel.py must be self-contained) ----
B, V, E, S, M, HOPS, C = 256, 50000, 512, 3, 200, 3, 7
D = S * E                   # 1536
NCORES = 8
BL = B // NCORES            # 32 batches per core
GB = 2                      # batches per gather group
NG = BL // GB               # 16 groups
NCO = BL // 2               # 16 batches per hop cohort
NGC = NG // 2               # 8 groups per cohort
NR = M + 1                  # 201 rows of the extended system [m; u0]
NIDX = (GB * NR + 127) // 128 * 128     # 512 gather indices per (group,slot)
NLO = NR - 128              # 73 rows in the low Gram block
NCOL = M + 8                # 208 cols: 200 attention scores + 8 F columns
NQUEUES = 4                 # SWDGE queues (ucode max)
SCALE = 64.0                # fp8 table scale; Gram lands in SCALE^2 units
SC2INV = float(2.0 ** -12)  # 1/SCALE^2, folded into the softmax exp

BF16 = ml_dtypes.bfloat16
FP8 = ml_dtypes.float8_e4m3

_CACHE = {}


def _position_encoding(sentence_size, embedding_size):
    i = np.arange(1, embedding_size + 1, dtype=np.float32)[:, None]
    j = np.arange(1, sentence_size + 1, dtype=np.float32)[None, :]
    le, ls = embedding_size + 1, sentence_size + 1
    enc = (i - (le - 1) / 2.0) * (j - (ls - 1) / 2.0)
    enc = 1.0 + 4.0 * enc / embedding_size / sentence_size
    return np.transpose(enc).astype(np.float32)


def _build_program(dpad):
    import concourse.bacc as bacc
    import concourse.bass as bass
    import concourse.mybir as mybir
    import concourse.tile as tile
    from concourse.masks import make_identity

    dt = mybir.dt
    nc = bacc.Bacc("TRN2", target_bir_lowering=False, debug=False,
                   num_swdge_queues=NQUEUES)

    emb_t = [
        nc.dram_tensor(f"emb{s}", [dpad, E], dt.float8e4, kind="ExternalInput")
        for s in range(S)
    ]
    idxm_t = nc.dram_tensor("idxm", [128, NG * S, NIDX // 16], dt.int16,
                            kind="ExternalInput")
    fcb_t = nc.dram_tensor("fcb", [NCO, 2, C], dt.float32,
                           kind="ExternalInput")
    e1m_t = nc.dram_tensor("e1m", [NLO, NCO * NCO], dt.bfloat16,
                           kind="ExternalInput")
    fh_t = nc.dram_tensor("fh", [128, BL * 8], dt.bfloat16,
                          kind="ExternalInput")
    fl_t = nc.dram_tensor("fl", [NLO, BL * 8], dt.bfloat16,
                          kind="ExternalInput")
    y_t = nc.dram_tensor("y", [BL, C], dt.float32, kind="ExternalOutput")

    with tile.TileContext(nc) as tc:
        with (
            tc.tile_pool(name="const", bufs=1) as cpool,
            tc.tile_pool(name="gath", bufs=4) as gpool,
            tc.tile_pool(name="gram", bufs=1) as grpool,
            tc.tile_pool(name="work", bufs=2) as wpool,
            tc.tile_pool(name="psum", bufs=2, space="PSUM") as ppool,
            tc.tile_pool(name="psT", bufs=1, space="PSUM") as tpool,
            tc.tile_pool(name="psS", bufs=1, space="PSUM") as spool,
        ):
            # ---- constants / small inputs ----
            idm = cpool.tile([128, NG * S, NIDX // 16], dt.int16)
            nc.sync.dma_start(idm[:], idxm_t[:])

            ScA = spool.tile([NCO, NCOL], dt.float32, tag="ScA")
            ScB = spool.tile([NCO, NCOL], dt.float32, tag="ScB")
            grh = grpool.tile([128, BL, NCOL], dt.bfloat16)
            grl = grpool.tile([NLO, BL, NCOL], dt.bfloat16)

            def issue_gathers(g):
                mts = []
                for s in range(S):
                    mt = gpool.tile([128, 4, NIDX], dt.float8e4, tag=f"mt{s}")
                    nc.gpsimd.dma_gather(
                        mt[:],
                        emb_t[s][:, :],
                        idm[:, g * S + s, :],
                        NIDX, GB * NR, E,
                        transpose=True,
                        queue_num=(g * S + s) % NQUEUES,
                    )
                    mts.append(mt)
                return mts

            # get group 0's gathers in flight before issuing anything else
            pend = issue_gathers(0)

            fcb = cpool.tile([NCO, 2, C], dt.float32)
            nc.sync.dma_start(fcb[:], fcb_t[:])
            ident = cpool.tile([32, 32], dt.bfloat16)
            make_identity(nc, ident[:])
            # e200 selector: [NLO, 16*16] with [72, j*17] = 1 -> stationary
            # operand that routes [G|F]_b[200, :] into scores row j.
            e1m = cpool.tile([NLO, NCO * NCO], dt.bfloat16)
            nc.sync.dma_start(e1m[:], e1m_t[:])
            # F values: contiguous DMA + strided DVE copy into the hop
            # operand (a strided dram->sbuf DMA decomposes into thousands
            # of 16B descriptors and poisons the rings).
            fhs = cpool.tile([128, BL * 8], dt.bfloat16)
            fls = cpool.tile([NLO, BL * 8], dt.bfloat16)
            nc.sync.dma_start(fhs[:], fh_t[:])
            nc.sync.dma_start(fls[:], fl_t[:])
            nc.vector.tensor_copy(
                grh[:, :, M:NCOL], fhs[:].rearrange("p (b f) -> p b f", f=8))
            nc.vector.tensor_copy(
                grl[:, :, M:NCOL], fls[:].rearrange("p (b f) -> p b f", f=8))

            def gram_group(g, mts, Sc):
                for b8 in range(GB):
                    bg = g * GB + b8
                    ph = ppool.tile([128, M], dt.float32, tag="ph")
                    pl = ppool.tile([NLO, M], dt.float32, tag="pl")
                    for s in range(S):
                        t = mts[s][:]
                        for k in range(2):
                            ki = 2 * s + k
                            off = t.offset + (b8 * NR) * 2 + k
                            lhsT_h = bass.AP(
                                t.tensor, off,
                                [t.ap[0], [2 * NIDX, 2], [2, 128]])
                            lhsT_l = bass.AP(
                                t.tensor, off + 256,
                                [t.ap[0], [2 * NIDX, 2], [2, NLO]])
                            rhs = bass.AP(
                                t.tensor, off,
                                [t.ap[0], [2 * NIDX, 2], [2, M]])
                            nc.tensor.matmul(
                                ph[:], lhsT=lhsT_h, rhs=rhs,
                                start=(ki == 0), stop=(ki == 5),
                                perf_mode=mybir.MatmulPerfMode.DoubleRow,
                            )
                            nc.tensor.matmul(
                                pl[:], lhsT=lhsT_l, rhs=rhs,
                                start=(ki == 0), stop=(ki == 5),
                                perf_mode=mybir.MatmulPerfMode.DoubleRow,
                            )
                    nc.scalar.copy(grh[:, bg, 0:M], ph[:])
                    nc.scalar.copy(grl[:, bg, 0:M], pl[:])
                    # fold the e200 init matmul into the Gram pipeline:
                    # scores row (bg % NCO) = [G|F]_bg[200, :].
                    j = bg % NCO
                    nc.tensor.matmul(
                        Sc[:], lhsT=e1m[:, j * NCO:(j + 1) * NCO],
                        rhs=grl[:, bg, :],
                        start=(j == 0), stop=False,
                        skip_group_check=True,
                    )

            def hop(Sc, base, h, tagp):
                last = h == HOPS - 1
                eexp = wpool.tile([NCO, M], dt.float32, tag="eexp" + tagp)
                sume = wpool.tile([NCO, 1], dt.float32, tag="sume" + tagp)
                nc.scalar.activation(
                    eexp[:], Sc[:, 0:M],
                    mybir.ActivationFunctionType.Exp,
                    scale=SC2INV,
                    accum_out=sume[:],
                )
                rs = wpool.tile([NCO, 1], dt.float32, tag="rs" + tagp)
                nc.vector.reciprocal(rs[:], sume[:])
                pbf = wpool.tile([NCO, M], dt.bfloat16, tag="pbf" + tagp)
                nc.vector.tensor_scalar_mul(pbf[:], eexp[:], rs[:])

                pth = tpool.tile([128, NCO], dt.bfloat16, tag="pth")
                ptl = tpool.tile([M - 128, NCO], dt.bfloat16, tag="ptl")
                nc.tensor.transpose(pth[:], pbf[:, 0:128],
                                    ident[0:NCO, 0:NCO])
                nc.tensor.transpose(ptl[:], pbf[:, 128:M],
                                    ident[0:NCO, 0:NCO])

                pm0 = wpool.tile([128, NCO * NCO], dt.bfloat16,
                                 tag="pm0" + tagp)
                pm1 = wpool.tile([NLO, NCO * NCO], dt.bfloat16,
                                 tag="pm1" + tagp)
                nc.vector.memset(pm0[:], 0.0)
                nc.vector.memset(pm1[:], 0.0)
                nc.vector.tensor_copy(pm0[:, ::NCO + 1], pth[:])
                nc.vector.tensor_copy(pm1[0:M - 128, ::NCO + 1], ptl[:])

                for j in range(NCO):
                    b = base + j
                    nc.tensor.matmul(
                        Sc[:], lhsT=pm0[:, j * NCO:(j + 1) * NCO],
                        rhs=grh[:, b, :],
                        start=False, stop=False, skip_group_check=True,
                    )
                    nc.tensor.matmul(
                        Sc[:], lhsT=pm1[:, j * NCO:(j + 1) * NCO],
                        rhs=grl[:, b, :],
                        start=False, stop=(last and j == NCO - 1),
                        skip_group_check=True,
                    )

            # ---- cohort A: groups 0..NGC-1, then its hops overlap B ----
            for g in range(NGC):
                mts = pend if g == 0 else issue_gathers(g)
                gram_group(g, mts, ScA)
            for h in range(HOPS):
                hop(ScA, 0, h, "a")
            ytA = wpool.tile([NCO, C], dt.float32, tag="ytA")
            nc.vector.tensor_add(ytA[:], ScA[:, M:M + C], fcb[:, 0, :])
            nc.sync.dma_start(y_t[0:NCO, :], ytA[:])

            # ---- cohort B ----
            for g in range(NGC, NG):
                gram_group(g, issue_gathers(g), ScB)
            for h in range(HOPS):
                hop(ScB, NCO, h, "b")
            ytB = wpool.tile([NCO, C], dt.float32, tag="ytB")
            nc.vector.tensor_add(ytB[:], ScB[:, M:M + C], fcb[:, 1, :])
            nc.sync.dma_start(y_t[NCO:BL, :], ytB[:])

    nc.compile()
    return nc


def _wrap16(lst):
    """Index list -> dma_gather layout: [16, n/16] with logical i at
    [i % 16, i // 16], replicated to 128 partitions."""
    a = np.asarray(lst, dtype=np.int16)
    assert a.size % 16 == 0
    a2 = a.reshape(-1, 16).T.copy()
    return np.tile(a2, (8, 1))


def _prepare_core_inputs(stories, queries, emb, fc_w, fc_b, enc):
    """Host-side shard prep: per-core token compaction + index layouts.

    Each per-slot table holds the enc-scaled, x64 fp8-quantized embedding
    rows for this core's tokens. The logits-path values F (= row @ fc_w.T)
    are precomputed per token in f32 (exact) and gathered on the host into
    small bf16 arrays loaded with a plain DMA."""
    per_core = []
    toks_list = []
    for cid in range(NCORES):
        st = stories[cid * BL:(cid + 1) * BL]
        qu = queries[cid * BL:(cid + 1) * BL]
        toks = np.unique(np.concatenate([st.ravel(), qu.ravel()]))
        toks_list.append(toks)
    dpad = max(len(t) for t in toks_list)
    dpad = (dpad + 127) // 128 * 128

    # full-vocab per-slot fp8 tables and exact F tables (vectorized)
    emb8 = []
    fs = []
    for s in range(S):
        sc = emb * enc[s * E:(s + 1) * E][None, :]
        emb8.append((sc * SCALE).astype(FP8))
        fs.append((sc @ fc_w[:, s * E:(s + 1) * E].T).astype(np.float32))

    fcb_rep = np.tile(fc_b[None, None, :], (NCO, 2, 1)).astype(np.float32)
    e1m = np.zeros((NLO, NCO * NCO), dtype=BF16)
    e1m[NR - 1 - 128, ::NCO + 1] = 1.0

    for cid in range(NCORES):
        st = stories[cid * BL:(cid + 1) * BL]     # (BL, M, S)
        qu = queries[cid * BL:(cid + 1) * BL]     # (BL, S)
        toks = toks_list[cid]
        ntok = len(toks)
        inv = np.zeros(V, dtype=np.int64)
        inv[toks] = np.arange(ntok)

        embs = []
        for s in range(S):
            tbl = np.zeros((dpad, E), dtype=FP8)
            tbl[:ntok] = emb8[s][toks]
            embs.append(tbl)

        sidx = inv[st]          # (BL, M, S)
        qidx = inv[qu]          # (BL, S)

        idxm = np.zeros((128, NG * S, NIDX // 16), dtype=np.int16)
        for g in range(NG):
            for s in range(S):
                # pad with -1: the SWDGE stops after the last valid index
                # (num_idxs_reg = GB*NR), skipping the pad descriptors.
                lst = np.full(NIDX, -1, dtype=np.int64)
                blk = lst[:GB * NR].reshape(GB, NR)
                blk[:, :M] = sidx[g * GB:(g + 1) * GB, :, s]
                blk[:, M] = qidx[g * GB:(g + 1) * GB, s]
                idxm[:, g * S + s, :] = _wrap16(lst)

        # F = [m; u0] @ fc_w.T per batch, exact f32 -> bf16, [row, BL, 8]
        fstory = sum(fs[s][st[:, :, s]] for s in range(S))   # (BL, M, C)
        fquery = sum(fs[s][qu[:, s]] for s in range(S))      # (BL, C)
        fh = np.zeros((128, BL, 8), dtype=BF16)
        fl = np.zeros((NLO, BL, 8), dtype=BF16)
        fh[:, :, :C] = fstory[:, 0:128, :].transpose(1, 0, 2)
        fl[0:M - 128, :, :C] = fstory[:, 128:M, :].transpose(1, 0, 2)
        fl[M - 128, :, :C] = fquery
        fh = fh.reshape(128, BL * 8)
        fl = fl.reshape(NLO, BL * 8)

        in_map = {
            "emb0": embs[0], "emb1": embs[1], "emb2": embs[2],
            "idxm": idxm, "fcb": fcb_rep, "e1m": e1m,
            "fh": fh, "fl": fl,
        }
        per_core.append(in_map)
    return dpad, per_core


def kernel(stories, queries, emb, fc_w, fc_b, _trace=False):
    from concourse import bass_utils

    stories = np.asarray(stories)
    queries = np.asarray(queries)
    emb = np.asarray(emb, dtype=np.float32)
    fc_w = np.asarray(fc_w, dtype=np.float32)
    fc_b = np.asarray(fc_b, dtype=np.float32)

    enc = _position_encoding(1, D).reshape(D)
    dpad, in_maps = _prepare_core_inputs(stories, queries, emb, fc_w, fc_b, enc)

    if _CACHE.get("dpad") != dpad:
        _CACHE["nc"] = _build_program(dpad)
        _CACHE["dpad"] = dpad
    nc = _CACHE["nc"]

    res = bass_utils.run_bass_kernel_spmd(
        nc, in_maps, core_ids=list(range(NCORES)), trace=_trace,
    )
    out = np.concatenate([r["y"] for r in res.results], axis=0)
    if _trace:
        _CACHE["last_exec_time_ns"] = res.exec_time_ns
        _CACHE["last_mean_exec_time_ns"] = res.mean_exec_time_ns
    return out.astype(np.float32)
